# revision 11
# baseline (speedup 1.0000x reference)
"""AttentionBlock kernel for 8 Trainium2 NeuronCores.

Reference computation (per batch element b of 8):
    xn  = GroupNorm(x, 32 groups, eps=1e-5) * gn_scale + gn_bias
    qkv = w_qkv @ xn + b_qkv          (1x1 conv == channel matmul)
    q, k, v = split(qkv)              each (C=256, N=4096)
    S   = (q^T k) * C^-0.5            (N, N) scores
    A   = softmax(S, axis=-1)
    AO  = (A @ v^T)^T                 (C, N)
    out = w_out @ AO + b_out + x

Sharding: data-parallel over batch - core i computes batch element i.

Per-core strategy (v2 - restructured from the 217.7us baseline):
  - GroupNorm is FOLDED INTO THE PROJECTION WEIGHTS: xn = a*x + beta per
    channel, so q = (Wq.diag(a))x + (bq + Wq.beta) etc.  No xn pass; the
    projections read fp8(x) cast piece-by-piece during the input DMA.
  - w_out is FOLDED INTO v: Wvo = w_out @ Wv (tiny 256^3 matmul), and the
    kernel projects vw = Wvo.xn directly.  The attention AO matmul then
    produces the FINAL projected output - no separate out-projection,
    no woutT, no ao_sb staging.
  - All projections run fp8 DoubleRow (contract 256 channels per matmul).
  - S^T tiles (key-part, query-free) at FD=512 into fp32 PSUM pair-tiles
    [128, 2, 512] (2 banks; TRN2 matmul PSUM output is fp32-only and one
    matmul output <= one 2KB bank).  Each exp ACTIVATE covers 1024 elems.
    ACT does exp ONLY (~147us, the 2nd wall); all evictions are on DVE.
  - AO chains are chunk-major and pipelined ONE BLOCK BEHIND the S/exp
    front: chains for query block b-1 interleave with the supertiles of
    block b, so the PE never waits on fresh exp output.  vw carries a
    ones-column so each chain also yields the softmax denominator.
  - Tail per chunk: recip (DVE), normalize to bf16 (DVE), 2 PE transposes,
    residual+bias add vs xb = x + (b_out + w_out@bv + Wvo@beta) (DVE), DMA.
Matmul inputs fp8 (fp32/bf16 PSUM); rel err vs fp32 ref ~1e-2 < 2e-2.
"""

import numpy as np

import concourse.bass as bass
import concourse.bacc as bacc
import concourse.mybir as mybir
import concourse.tile as tile
from concourse.bass_utils import run_bass_kernel_spmd
from concourse.masks import make_identity

F32 = mybir.dt.float32
BF16 = mybir.dt.bfloat16
FP8 = mybir.dt.float8e4
ESC_BIAS = -3.4657359027997265  # ln(1/32): exp scaled into fp8e4m3 range
VPAD = 16                       # vw free-dim pad so the DR middle step %16==0

B = 8          # batch / cores
C = 256        # channels
P = 128        # partitions
CK = C // P    # channel chunks (2)
H = W = 64
N = H * W      # 4096 spatial positions
NB = 512       # query-block width
NBLK = N // NB  # 8 query blocks
MT = N // P    # 32 key tiles of 128
G = 32         # groups
GS = C // G    # channels per group (8)
EPS = 1e-5
SCALE = float(C) ** -0.5
O_QKV = 3 * C  # 768
BN_SUB = 512   # bn_stats subgroup width
NSUP = 2       # key tiles per S supertile (exp batch = NSUP*NB elements;
               # capped at 2: TRN2 matmul PSUM output is fp32-only and one
               # matmul output must fit one 2KB bank -> [P, 2, 512] f32)


_TILE_FREES = []


def _tile(tc, *args, **kwargs):
    t, free = tc.tile(*args, **kwargs)
    _TILE_FREES.append(free)  # keep persistent tiles alive (GC would release)
    return t


def build_attention_program(nc, n=N):
    """Emit the full single-core program into `nc` (one batch element)."""
    nblk = max(1, n // NB)       # query blocks
    mtn = n // P                 # key tiles
    nbsub = min(n, BN_SUB)       # bn_stats subgroup width
    nsup = min(NSUP, mtn)        # key tiles per supertile
    nst = mtn // nsup            # supertiles per block (8)
    GL = P // GS                 # groups per channel chunk (16)
    x_d = nc.dram_tensor("x", [C, n], F32, kind="ExternalInput").ap()
    gns_d = nc.dram_tensor("gn_scale", [C], F32, kind="ExternalInput").ap()
    gnb_d = nc.dram_tensor("gn_bias", [C], F32, kind="ExternalInput").ap()
    wqkv_d = nc.dram_tensor("w_qkv", [O_QKV, C], F32, kind="ExternalInput").ap()
    bqkv_d = nc.dram_tensor("b_qkv", [O_QKV], F32, kind="ExternalInput").ap()
    wout_d = nc.dram_tensor("w_out", [C, C], F32, kind="ExternalInput").ap()
    bout_d = nc.dram_tensor("b_out", [C], F32, kind="ExternalInput").ap()
    out_d = nc.dram_tensor("out", [C, n], F32, kind="ExternalOutput").ap()

    DR = mybir.MatmulPerfMode.DoubleRow
    EXPF = mybir.ActivationFunctionType.Exp

    with tile.TileContext(nc) as tc:
        # ---------------- persistent SBUF ----------------
        x_sb = [_tile(tc, [P, n], F32, name=f"x_sb{j}") for j in range(CK)]
        xb_sb = [_tile(tc, [P, n], F32, name=f"xb_sb{j}") for j in range(CK)]
        xf8 = _tile(tc, [P, CK, n], FP8, name="xf8")
        q3 = _tile(tc, [P, CK, n], FP8, name="q3")
        k3 = _tile(tc, [P, CK, n], FP8, name="k3")
        vw_sb = _tile(tc, [P, mtn, C + VPAD], FP8, name="vw_sb")
        expT = [_tile(tc, [P, mtn, NB], FP8, name=f"expT{i}") for i in range(2)]
        # weight stages (bf16, pre-GN-fold) and final fp8 DR layouts
        wqkT_st = [_tile(tc, [P, 2 * C], BF16, name=f"wqkTst{j}") for j in range(CK)]
        woT_st = [_tile(tc, [P, C], BF16, name=f"woTst{j}") for j in range(CK)]
        wvoT_st = [_tile(tc, [P, C], BF16, name=f"wvoTst{j}") for j in range(CK)]
        wqkT3 = _tile(tc, [P, CK, 2 * C], FP8, name="wqkT3")
        wvoT3 = _tile(tc, [P, CK, C], FP8, name="wvoT3")
        ident_f = _tile(tc, [P, P], F32, name="ident_f")
        ident_b = _tile(tc, [P, P], BF16, name="ident_b")

        # small per-channel vectors
        scale_sb = [_tile(tc, [P, 1], F32, name=f"scale_sb{j}") for j in range(CK)]
        bias_sb = [_tile(tc, [P, 1], F32, name=f"bias_sb{j}") for j in range(CK)]
        bq_sb = [_tile(tc, [P, 1], F32, name=f"bq_sb{j}") for j in range(CK)]
        bk_sb = [_tile(tc, [P, 1], F32, name=f"bk_sb{j}") for j in range(CK)]
        bv_sb = [_tile(tc, [P, 1], F32, name=f"bv_sb{j}") for j in range(CK)]
        bo_sb = [_tile(tc, [P, 1], F32, name=f"bo_sb{j}") for j in range(CK)]
        a_c = [_tile(tc, [P, 1], F32, name=f"a_c{j}") for j in range(CK)]
        beta_c = [_tile(tc, [P, 1], F32, name=f"beta_c{j}") for j in range(CK)]
        beta_b16 = [_tile(tc, [P, 1], BF16, name=f"beta_b{j}") for j in range(CK)]
        bv_b16 = [_tile(tc, [P, 1], BF16, name=f"bv_b{j}") for j in range(CK)]
        bqs_sb = [_tile(tc, [P, 1], F32, name=f"bqs{j}") for j in range(CK)]
        bks_sb = [_tile(tc, [P, 1], F32, name=f"bks{j}") for j in range(CK)]
        tb_sb = [_tile(tc, [P, 1], F32, name=f"tb{j}") for j in range(CK)]
        sel = [_tile(tc, [P, GL], F32, name=f"sel{j}") for j in range(CK)]
        selT = [_tile(tc, [GL, P], F32, name=f"selT{j}") for j in range(CK)]
        eps_sb = _tile(tc, [GL, 1], F32, name="eps_sb")
        eln_sb = _tile(tc, [P, 1], F32, name="eln_sb")

        from contextlib import ExitStack

        # =========================================================
        # PREAMBLE (own psum/sbuf pool scope, released before attention)
        # =========================================================
        with ExitStack() as pre:
            ps_pre = pre.enter_context(tc.tile_pool(name="ps_pre", bufs=4, space="PSUM"))
            work = pre.enter_context(tc.tile_pool(name="work", bufs=3))

            # ---------------- input DMA + per-piece cast/stats ----------
            npieces = max(1, n // BN_SUB)
            pw = n // npieces
            stats = [work.tile([P, npieces, 6], F32, tag=f"bnst{j}",
                               name=f"bnst{j}") for j in range(CK)]
            for piece in range(npieces):
                for j in range(CK):
                    sl = slice(piece * pw, (piece + 1) * pw)
                    nc.sync.dma_start(
                        out=x_sb[j][:, sl],
                        in_=x_d[j * P:(j + 1) * P, sl],
                    )
                    nc.vector.bn_stats(out=stats[j][:, piece, :],
                                       in_=x_sb[j][:, sl])
                    nc.vector.tensor_copy(xf8[:, j, sl], x_sb[j][:, sl])
            wq_raw = []
            for i in range(4):  # q,k weight rows 0..512
                t = work.tile([P, C], F32, tag="wraw", name=f"wqraw{i}")
                nc.sync.dma_start(out=t, in_=wqkv_d[i * P:(i + 1) * P, :])
                wq_raw.append(t)
            wv_raw = []
            for i in range(CK):  # v weight rows 512..768
                t = work.tile([P, C], F32, tag="wvraw", name=f"wvraw{i}")
                nc.sync.dma_start(out=t, in_=wqkv_d[2 * C + i * P:2 * C + (i + 1) * P, :])
                wv_raw.append(t)
            wo_raw = []
            for i in range(CK):
                t = work.tile([P, C], F32, tag="woraw", name=f"woraw{i}")
                nc.sync.dma_start(out=t, in_=wout_d[i * P:(i + 1) * P, :])
                wo_raw.append(t)
            for j in range(CK):
                sl = slice(j * P, (j + 1) * P)
                nc.sync.dma_start(out=scale_sb[j], in_=gns_d[sl].rearrange("(a u) -> a u", u=1))
                nc.sync.dma_start(out=bias_sb[j], in_=gnb_d[sl].rearrange("(a u) -> a u", u=1))
                nc.sync.dma_start(out=bq_sb[j], in_=bqkv_d[sl].rearrange("(a u) -> a u", u=1))
                nc.sync.dma_start(
                    out=bk_sb[j],
                    in_=bqkv_d[C + j * P:C + (j + 1) * P].rearrange("(a u) -> a u", u=1))
                nc.sync.dma_start(
                    out=bv_sb[j],
                    in_=bqkv_d[2 * C + j * P:2 * C + (j + 1) * P].rearrange("(a u) -> a u", u=1))
                nc.sync.dma_start(out=bo_sb[j], in_=bout_d[sl].rearrange("(a u) -> a u", u=1))

            # ---------------- constants ----------------
            make_identity(nc, ident_f)
            make_identity(nc, ident_b)
            nc.vector.memset(eps_sb, EPS)
            nc.vector.memset(eln_sb, ESC_BIAS)
            nc.gpsimd.memset(vw_sb[:, :, C:C + VPAD], 0.0)
            nc.gpsimd.memset(vw_sb[:, :, C:C + 1], 1.0)
            # per-chunk local selectors: sel[c, g] = 1/GS where c//GS == g
            for j in range(CK):
                nc.gpsimd.memset(sel[j], 0.0)
                nc.gpsimd.affine_select(
                    out=sel[j], in_=sel[j], compare_op=mybir.AluOpType.is_gt,
                    fill=1.0 / GS, base=1 - GS, pattern=[[-GS, GL]],
                    channel_multiplier=1,
                )
                nc.gpsimd.affine_select(
                    out=sel[j], in_=sel[j], compare_op=mybir.AluOpType.is_ge,
                    fill=0.0, base=0, pattern=[[-GS, GL]], channel_multiplier=1,
                )
                nc.gpsimd.memset(selT[j], 0.0)
                nc.gpsimd.affine_select(
                    out=selT[j], in_=selT[j], compare_op=mybir.AluOpType.is_gt,
                    fill=1.0, base=1 - GS, pattern=[[1, P]], channel_multiplier=-GS,
                )
                nc.gpsimd.affine_select(
                    out=selT[j], in_=selT[j], compare_op=mybir.AluOpType.is_ge,
                    fill=0.0, base=0, pattern=[[1, P]], channel_multiplier=-GS,
                )

            # ------------- weight transposes (PE) + bf16 staging ---------
            for i in range(4):
                for j in range(CK):
                    pt = ps_pre.tile([P, P], F32, tag="p", name="wtp")
                    nc.tensor.transpose(pt, wq_raw[i][:, j * P:(j + 1) * P], ident_f)
                    nc.vector.tensor_copy(wqkT_st[j][:, i * P:(i + 1) * P], pt)
            for i in range(CK):
                for j in range(CK):
                    pt = ps_pre.tile([P, P], F32, tag="p", name="wtp2")
                    nc.tensor.transpose(pt, wo_raw[i][:, j * P:(j + 1) * P], ident_f)
                    nc.vector.tensor_copy(woT_st[j][:, i * P:(i + 1) * P], pt)
            wv_b = []
            for i in range(CK):
                t = work.tile([P, C], BF16, tag="wvb", name=f"wvb{i}")
                nc.vector.tensor_copy(t, wv_raw[i])
                wv_b.append(t)

            # ------------- Wvo = w_out @ Wv  (bf16, PE) ------------------
            wvo_st = []
            for os_ in range(CK):
                ps = ps_pre.tile([P, C], F32, tag="p", name="ps_wvo")
                for mc in range(CK):
                    nc.tensor.matmul(
                        ps, woT_st[mc][:, os_ * P:(os_ + 1) * P], wv_b[mc],
                        start=(mc == 0), stop=(mc == CK - 1),
                    )
                t = work.tile([P, C], BF16, tag="wvo_sb", name=f"wvo{os_}")
                nc.vector.tensor_copy(t, ps)
                wvo_st.append(t)
            for os_ in range(CK):  # WvoT (c-part, o-free) via PE transpose
                for j in range(CK):
                    pt = ps_pre.tile([P, P], BF16, tag="p", name="wvtp")
                    nc.tensor.transpose(pt, wvo_st[os_][:, j * P:(j + 1) * P], ident_b)
                    nc.vector.tensor_copy(wvoT_st[j][:, os_ * P:(os_ + 1) * P], pt)

            # ------------- group norm stats -> a_c, beta_c ---------------
            for j in range(CK):
                mv = work.tile([P, 2], F32, tag="mv", name=f"mv{j}")
                nc.vector.bn_aggr(out=mv, in_=stats[j])
                m2 = work.tile([P, 1], F32, tag="m2", name=f"m2{j}")
                nc.vector.tensor_mul(m2, mv[:, 0:1], mv[:, 0:1])
                mv2 = work.tile([P, 2], F32, tag="mv2", name=f"mv2{j}")
                nc.vector.tensor_copy(mv2[:, 0:1], mv[:, 0:1])
                nc.vector.tensor_add(mv2[:, 1:2], mv[:, 1:2], m2)
                ps_g = ps_pre.tile([GL, 2], F32, tag="p", name="ps_g")
                nc.tensor.matmul(ps_g, sel[j], mv2, start=True, stop=True)
                gs = work.tile([GL, 2], F32, tag="gs", name=f"gs{j}")
                nc.vector.tensor_copy(gs, ps_g)
                gm2 = work.tile([GL, 1], F32, tag="gm2", name=f"gm2{j}")
                nc.vector.tensor_mul(gm2, gs[:, 0:1], gs[:, 0:1])
                gvar = work.tile([GL, 1], F32, tag="gvar", name=f"gvar{j}")
                nc.vector.tensor_sub(gvar, gs[:, 1:2], gm2)
                gsd = work.tile([GL, 1], F32, tag="gsd", name=f"gsd{j}")
                nc.scalar.activation(out=gsd, in_=gvar,
                                     func=mybir.ActivationFunctionType.Sqrt,
                                     bias=eps_sb, scale=1.0)
                grstd = work.tile([GL, 1], F32, tag="grstd", name=f"grstd{j}")
                nc.vector.reciprocal(grstd, gsd)
                gstat2 = work.tile([GL, 2], F32, tag="gstat2", name=f"gstat2{j}")
                nc.vector.tensor_copy(gstat2[:, 0:1], gs[:, 0:1])
                nc.vector.tensor_copy(gstat2[:, 1:2], grstd)
                ps_bc = ps_pre.tile([P, 2], F32, tag="p", name="ps_bc")
                nc.tensor.matmul(ps_bc, selT[j], gstat2, start=True, stop=True)
                nc.vector.tensor_mul(a_c[j], ps_bc[:, 1:2], scale_sb[j])
                t_c = work.tile([P, 1], F32, tag="t_c", name=f"t_c{j}")
                nc.vector.tensor_mul(t_c, ps_bc[:, 0:1], a_c[j])
                nc.vector.tensor_sub(beta_c[j], bias_sb[j], t_c)
                nc.vector.tensor_copy(beta_b16[j], beta_c[j])
                nc.vector.tensor_copy(bv_b16[j], bv_sb[j])

            # preload the exp table set so the first real exp doesn't stall
            dummy_exp = work.tile([1, 1], F32, tag="dummy", name="dummy_exp")
            nc.scalar.activation(out=dummy_exp, in_=eps_sb[0:1, :], func=EXPF)

            # ------------- bias folds (tiny PE matmuls) ------------------
            # bq' = bq + Wq@beta ; bk' = bk + Wk@beta
            for idx, (dst, bsrc) in enumerate([(bqs_sb, bq_sb), (bks_sb, bk_sb)]):
                for oc in range(CK):
                    ps = ps_pre.tile([P, 1], F32, tag="p", name="ps_bias")
                    for kc in range(CK):
                        off = idx * C + oc * P
                        nc.tensor.matmul(
                            ps, wqkT_st[kc][:, off:off + P], beta_b16[kc],
                            start=(kc == 0), stop=(kc == CK - 1),
                        )
                    nc.vector.tensor_add(dst[oc], ps, bsrc[oc])
            for oc in range(CK):
                nc.vector.tensor_scalar_mul(out=bqs_sb[oc], in0=bqs_sb[oc],
                                            scalar1=SCALE)
            # totbias = b_out + w_out@bv + Wvo@beta
            for oc in range(CK):
                ps1 = ps_pre.tile([P, 1], F32, tag="p", name="ps_tb1")
                for kc in range(CK):
                    nc.tensor.matmul(
                        ps1, wvoT_st[kc][:, oc * P:(oc + 1) * P], beta_b16[kc],
                        start=(kc == 0), stop=(kc == CK - 1),
                    )
                ps2 = ps_pre.tile([P, 1], F32, tag="p", name="ps_tb2")
                for mc in range(CK):
                    nc.tensor.matmul(
                        ps2, woT_st[mc][:, oc * P:(oc + 1) * P], bv_b16[mc],
                        start=(mc == 0), stop=(mc == CK - 1),
                    )
                t1 = work.tile([P, 1], F32, tag="tb1", name="tb1")
                nc.vector.tensor_copy(t1, ps1)
                nc.vector.tensor_add(t1, t1, ps2)
                nc.vector.tensor_add(tb_sb[oc], t1, bo_sb[oc])

            # ------------- GN-folded fp8 weights -------------------------
            for kc in range(CK):
                nc.vector.tensor_scalar_mul(out=wqkT3[:, kc, :],
                                            in0=wqkT_st[kc], scalar1=a_c[kc])
                nc.vector.tensor_scalar_mul(out=wvoT3[:, kc, :],
                                            in0=wvoT_st[kc], scalar1=a_c[kc])

            # ------------- xb = x + totbias (residual + folded bias) -----
            for j in range(CK):
                nc.vector.tensor_scalar_add(out=xb_sb[j], in0=x_sb[j],
                                            scalar1=tb_sb[j])

            # ------------- q, k projections (fp8 DR, FD=512) -------------
            qn = min(n, NB)
            for idx, (dst, sc, bias_ap) in enumerate(
                [(k3, 1.0, bks_sb), (q3, SCALE, bqs_sb)]
            ):
                for nb in range(n // qn):
                    nsl = slice(nb * qn, (nb + 1) * qn)
                    for oc in range(CK):
                        off = (1 - idx) * C + oc * P  # k first, then q
                        ps = ps_pre.tile([P, qn], F32, tag="p", name="ps_qk")
                        nc.tensor.matmul(
                            ps, wqkT3[:, :, off:off + P], xf8[:, :, nsl],
                            perf_mode=DR, start=True, stop=True,
                        )
                        nc.vector.tensor_scalar(
                            out=dst[:, oc, nsl], in0=ps,
                            scalar1=sc, scalar2=bias_ap[oc],
                            op0=mybir.AluOpType.mult, op1=mybir.AluOpType.add,
                        )

        # =========================================================
        # ATTENTION (software-pipelined: chains lag one block)
        # =========================================================
        with ExitStack() as att:
            ps_s = att.enter_context(tc.tile_pool(name="ps_s", bufs=2, space="PSUM"))
            ps_ao = att.enter_context(tc.tile_pool(name="ps_ao", bufs=2, space="PSUM"))
            ps_t = att.enter_context(tc.tile_pool(name="ps_t", bufs=2, space="PSUM"))
            evac = att.enter_context(tc.tile_pool(name="evac", bufs=2))
            work2 = att.enter_context(tc.tile_pool(name="work2", bufs=3))

            def emit_super(b, st):
                par = b % 2
                ps = ps_s.tile([P, nsup, NB], F32, tag="s", name="ps_s")
                for sub in range(nsup):
                    t = nsup * st + sub
                    nc.tensor.matmul(
                        ps[:, sub, :],
                        k3[:, :, t * P:(t + 1) * P],
                        q3[:, :, b * NB:(b + 1) * NB],
                        perf_mode=DR, start=True, stop=True,
                    )
                nc.scalar.activation(
                    out=expT[par][:, nsup * st:nsup * (st + 1), :], in_=ps,
                    func=EXPF, bias=eln_sb,
                )

            def emit_vw(mt):
                ps = ps_t.tile([P, C], F32, tag="t", name="ps_vw")
                nc.tensor.matmul(
                    ps, xf8[:, :, mt * P:(mt + 1) * P], wvoT3[:, :, :],
                    perf_mode=DR, start=True, stop=True,
                )
                nc.vector.tensor_copy(vw_sb[:, mt, 0:C], ps)

            def emit_chain(b, c):
                par = b % 2
                ao = ps_ao.tile([P, NB], F32, tag="ao", name="ps_ao")
                npairs = mtn // 2
                for j in range(npairs):
                    nc.tensor.matmul(
                        ao[:, 0:C + VPAD],
                        expT[par][:, 2 * j:2 * j + 2, c * P:(c + 1) * P],
                        vw_sb[:, 2 * j:2 * j + 2, :],
                        perf_mode=DR, start=(j == 0), stop=(j == npairs - 1),
                    )
                return ao

            def emit_tail(b, c, ao, o_tiles):
                recip = work2.tile([P, 1], F32, tag="recip", name="recip")
                nc.vector.reciprocal(recip, ao[:, C:C + 1])
                aot = work2.tile([P, C], BF16, tag="aot", bufs=4, name="aot")
                nc.vector.tensor_scalar_mul(out=aot, in0=ao[:, 0:C], scalar1=recip)
                for oc in range(CK):
                    pt = ps_t.tile([P, P], BF16, tag="t", name="ao_tp")
                    nc.tensor.transpose(pt, aot[:, oc * P:(oc + 1) * P], ident_b)
                    nc.vector.tensor_add(
                        o_tiles[oc][:, c * P:(c + 1) * P], pt,
                        xb_sb[oc][:, b * NB + c * P:b * NB + (c + 1) * P],
                    )

            def emit_out_dma(b, o_tiles):
                nsl = slice(b * NB, (b + 1) * NB)
                for oc in range(CK):
                    nc.sync.dma_start(out=out_d[oc * P:(oc + 1) * P, nsl],
                                      in_=o_tiles[oc])

            nsubs = NB // P  # 4 query chunks per block
            prev = None
            vw_left = list(range(mtn))  # vw tiles to emit inside block 0
            for b in range(nblk):
                chains = {}
                o_tiles = None
                if prev is not None:
                    o_tiles = [evac.tile([P, NB], F32, tag=f"o{oc}",
                                         name=f"o_sb{oc}") for oc in range(CK)]
                for st in range(nst):
                    emit_super(b, st)
                    if prev is None:
                        # fill the ACT-paced warmup block with the vw projection
                        for _ in range(min(2, len(vw_left))):
                            emit_vw(vw_left.pop(0))
                    else:
                        # interleave: chains early, tails trailing (16 slots)
                        sched = {1: ('c', 0), 3: ('c', 1), 5: ('t', 0),
                                 7: ('c', 2), 9: ('t', 1), 11: ('c', 3),
                                 13: ('t', 2), 15: ('t', 3)}
                        if st in sched:
                            kind, c = sched[st]
                            if kind == 'c':
                                chains[c] = emit_chain(prev, c)
                            else:
                                emit_tail(prev, c, chains[c], o_tiles)
                if prev is not None:
                    emit_out_dma(prev, o_tiles)
                prev, prev_chains = b, chains
            # drain: chains + tails for the final block
            o_tiles = [evac.tile([P, NB], F32, tag=f"o{oc}", name=f"o_sb{oc}")
                       for oc in range(CK)]
            chains = {}
            for c in range(nsubs):
                chains[c] = emit_chain(prev, c)
                if c >= 1:
                    emit_tail(prev, c - 1, chains[c - 1], o_tiles)
            emit_tail(prev, nsubs - 1, chains[nsubs - 1], o_tiles)
            emit_out_dma(prev, o_tiles)

    return nc


_CACHED_NC = {}


def build_nc(n=N):
    if n not in _CACHED_NC:
        nc = bacc.Bacc("TRN2", target_bir_lowering=False, debug=False,
                       num_devices=B)
        build_attention_program(nc, n=n)
        nc.compile()
        _CACHED_NC[n] = nc
    return _CACHED_NC[n]


def make_in_maps(x, gn_scale, gn_bias, w_qkv, b_qkv, w_out, b_out):
    f = np.ascontiguousarray
    return [
        {
            "x": f(x[b].reshape(C, N), dtype=np.float32),
            "gn_scale": f(gn_scale, dtype=np.float32),
            "gn_bias": f(gn_bias, dtype=np.float32),
            "w_qkv": f(w_qkv, dtype=np.float32),
            "b_qkv": f(b_qkv, dtype=np.float32),
            "w_out": f(w_out, dtype=np.float32),
            "b_out": f(b_out, dtype=np.float32),
        }
        for b in range(B)
    ]


def kernel(x, gn_scale, gn_bias, w_qkv, b_qkv, w_out, b_out, _trace=False,
           _tmpdir=None):
    x = np.asarray(x)
    nc = build_nc()
    in_maps = make_in_maps(x, gn_scale, gn_bias, w_qkv, b_qkv, w_out, b_out)
    res = run_bass_kernel_spmd(nc, in_maps, list(range(B)), trace=_trace,
                               tmpdir=_tmpdir)
    out = np.stack([res.results[b]["out"] for b in range(B)])
    out = out.reshape(B, C, H, W).astype(np.float32)
    if _trace:
        kernel.last_exec_time_ns = res.exec_time_ns
        kernel.last_results = res
    return out


# revision 15
# speedup vs baseline: 1.0315x; 1.0315x over previous
"""AttentionBlock kernel for 8 Trainium2 NeuronCores.

Reference computation (per batch element b of 8):
    xn  = GroupNorm(x, 32 groups, eps=1e-5) * gn_scale + gn_bias
    qkv = w_qkv @ xn + b_qkv          (1x1 conv == channel matmul)
    q, k, v = split(qkv)              each (C=256, N=4096)
    S   = (q^T k) * C^-0.5            (N, N) scores
    A   = softmax(S, axis=-1)
    AO  = (A @ v^T)^T                 (C, N)
    out = w_out @ AO + b_out + x

Sharding: data-parallel over batch - core i computes batch element i.

Per-core strategy (v2 - restructured from the 217.7us baseline):
  - GroupNorm is FOLDED INTO THE PROJECTION WEIGHTS: xn = a*x + beta per
    channel, so q = (Wq.diag(a))x + (bq + Wq.beta) etc.  No xn pass; the
    projections read fp8(x) cast piece-by-piece during the input DMA.
  - w_out is FOLDED INTO v: Wvo = w_out @ Wv (tiny 256^3 matmul), and the
    kernel projects vw = Wvo.xn directly.  The attention AO matmul then
    produces the FINAL projected output - no separate out-projection,
    no woutT, no ao_sb staging.
  - All projections run fp8 DoubleRow (contract 256 channels per matmul).
  - S^T tiles (key-part, query-free) at FD=512 into fp32 PSUM pair-tiles
    [128, 2, 512] (2 banks; TRN2 matmul PSUM output is fp32-only and one
    matmul output <= one 2KB bank).  Each exp ACTIVATE covers 1024 elems.
    ACT does exp ONLY (~147us, the 2nd wall); all evictions are on DVE.
  - AO chains are chunk-major and pipelined ONE BLOCK BEHIND the S/exp
    front: chains for query block b-1 interleave with the supertiles of
    block b, so the PE never waits on fresh exp output.  vw carries a
    ones-column so each chain also yields the softmax denominator.
  - Tail per chunk: recip (DVE), normalize to bf16 (DVE), 2 PE transposes,
    residual+bias add vs xb = x + (b_out + w_out@bv + Wvo@beta) (DVE), DMA.
Matmul inputs fp8 (fp32/bf16 PSUM); rel err vs fp32 ref ~1e-2 < 2e-2.
"""

import numpy as np

import concourse.bass as bass
import concourse.bacc as bacc
import concourse.mybir as mybir
import concourse.tile as tile
from concourse.bass_utils import run_bass_kernel_spmd
from concourse.masks import make_identity

F32 = mybir.dt.float32
BF16 = mybir.dt.bfloat16
FP8 = mybir.dt.float8e4
ESC_BIAS = -3.4657359027997265  # ln(1/32): exp into fp8e4m3 range. 1/8
                                # overflows: |S| reaches ~9.5 and exp(9)/8
                                # > 448 = fp8 inf -> NaN after normalize.
VPAD = 16                       # vw free-dim pad so the DR middle step %16==0

B = 8          # batch / cores
C = 256        # channels
P = 128        # partitions
CK = C // P    # channel chunks (2)
H = W = 64
N = H * W      # 4096 spatial positions
NB = 512       # query-block width
NBLK = N // NB  # 8 query blocks
MT = N // P    # 32 key tiles of 128
G = 32         # groups
GS = C // G    # channels per group (8)
EPS = 1e-5
SCALE = float(C) ** -0.5
O_QKV = 3 * C  # 768
BN_SUB = 512   # bn_stats subgroup width
NSUP = 2       # key tiles per S supertile ([P,2,512] f32 = 2 banks; PSUM:
               # 2 bufs x 2 (S) + 2 (AO chains) + 2 (transposes) = 8 banks)


_TILE_FREES = []


def _tile(tc, *args, **kwargs):
    t, free = tc.tile(*args, **kwargs)
    _TILE_FREES.append(free)  # keep persistent tiles alive (GC would release)
    return t


def build_attention_program(nc, n=N):
    """Emit the full single-core program into `nc` (one batch element)."""
    nblk = max(1, n // NB)       # query blocks
    mtn = n // P                 # key tiles
    nbsub = min(n, BN_SUB)       # bn_stats subgroup width
    nsup = min(NSUP, mtn)        # key tiles per supertile
    nst = mtn // nsup            # supertiles per block (8)
    GL = P // GS                 # groups per channel chunk (16)
    x_d = nc.dram_tensor("x", [C, n], F32, kind="ExternalInput").ap()
    gns_d = nc.dram_tensor("gn_scale", [C], F32, kind="ExternalInput").ap()
    gnb_d = nc.dram_tensor("gn_bias", [C], F32, kind="ExternalInput").ap()
    wqkv_d = nc.dram_tensor("w_qkv", [O_QKV, C], F32, kind="ExternalInput").ap()
    bqkv_d = nc.dram_tensor("b_qkv", [O_QKV], F32, kind="ExternalInput").ap()
    wout_d = nc.dram_tensor("w_out", [C, C], F32, kind="ExternalInput").ap()
    bout_d = nc.dram_tensor("b_out", [C], F32, kind="ExternalInput").ap()
    out_d = nc.dram_tensor("out", [C, n], F32, kind="ExternalOutput").ap()

    DR = mybir.MatmulPerfMode.DoubleRow
    EXPF = mybir.ActivationFunctionType.Exp

    with tile.TileContext(nc) as tc:
        # ---------------- persistent SBUF ----------------
        x_sb = [_tile(tc, [P, n], F32, name=f"x_sb{j}") for j in range(CK)]
        xb_sb = [_tile(tc, [P, n], F32, name=f"xb_sb{j}") for j in range(CK)]
        xf8 = _tile(tc, [P, CK, n], FP8, name="xf8")
        q3 = _tile(tc, [P, CK, n], FP8, name="q3")
        k3 = _tile(tc, [P, CK, n], FP8, name="k3")
        vw_sb = _tile(tc, [P, mtn, C + VPAD], FP8, name="vw_sb")
        expT = [_tile(tc, [P, mtn, NB], FP8, name=f"expT{i}") for i in range(2)]
        # weight stages (bf16, pre-GN-fold) and final fp8 DR layouts
        wqkT_st = [_tile(tc, [P, 2 * C], BF16, name=f"wqkTst{j}") for j in range(CK)]
        woT_st = [_tile(tc, [P, C], BF16, name=f"woTst{j}") for j in range(CK)]
        wvoT_st = [_tile(tc, [P, C], BF16, name=f"wvoTst{j}") for j in range(CK)]
        wqkT3 = _tile(tc, [P, CK, 2 * C], FP8, name="wqkT3")
        wvoT3 = _tile(tc, [P, CK, C], FP8, name="wvoT3")
        ident_f = _tile(tc, [P, P], F32, name="ident_f")
        ident_b = _tile(tc, [P, P], BF16, name="ident_b")

        # small per-channel vectors
        scale_sb = [_tile(tc, [P, 1], F32, name=f"scale_sb{j}") for j in range(CK)]
        bias_sb = [_tile(tc, [P, 1], F32, name=f"bias_sb{j}") for j in range(CK)]
        bq_sb = [_tile(tc, [P, 1], F32, name=f"bq_sb{j}") for j in range(CK)]
        bk_sb = [_tile(tc, [P, 1], F32, name=f"bk_sb{j}") for j in range(CK)]
        bv_sb = [_tile(tc, [P, 1], F32, name=f"bv_sb{j}") for j in range(CK)]
        bo_sb = [_tile(tc, [P, 1], F32, name=f"bo_sb{j}") for j in range(CK)]
        a_c = [_tile(tc, [P, 1], F32, name=f"a_c{j}") for j in range(CK)]
        beta_c = [_tile(tc, [P, 1], F32, name=f"beta_c{j}") for j in range(CK)]
        beta_b16 = [_tile(tc, [P, 1], BF16, name=f"beta_b{j}") for j in range(CK)]
        bv_b16 = [_tile(tc, [P, 1], BF16, name=f"bv_b{j}") for j in range(CK)]
        bqs_sb = [_tile(tc, [P, 1], F32, name=f"bqs{j}") for j in range(CK)]
        bks_sb = [_tile(tc, [P, 1], F32, name=f"bks{j}") for j in range(CK)]
        tb_sb = [_tile(tc, [P, 1], F32, name=f"tb{j}") for j in range(CK)]
        sel = [_tile(tc, [P, GL], F32, name=f"sel{j}") for j in range(CK)]
        selT = [_tile(tc, [GL, P], F32, name=f"selT{j}") for j in range(CK)]
        eps_sb = _tile(tc, [GL, 1], F32, name="eps_sb")
        eln_sb = _tile(tc, [P, 1], F32, name="eln_sb")

        from contextlib import ExitStack

        with ExitStack() as ctx:
            # two PSUM rings cover everything: ps_s 2x3 banks (S supertiles;
            # preamble qk psums alternate here) + ps_ao 2x1 banks (AO chains;
            # preamble transposes/GN/bias and block-0 vw psums reuse it).
            ps_s = ctx.enter_context(tc.tile_pool(name="ps_s", bufs=2, space="PSUM"))
            ps_ao = ctx.enter_context(tc.tile_pool(name="ps_ao", bufs=2, space="PSUM"))
            ps_t = ctx.enter_context(tc.tile_pool(name="ps_t", bufs=2, space="PSUM"))
            work = ctx.enter_context(tc.tile_pool(name="work", bufs=3))
            evac = ctx.enter_context(tc.tile_pool(name="evac", bufs=3))
            work2 = ctx.enter_context(tc.tile_pool(name="work2", bufs=3))

            # ---------------- input DMA + per-piece cast/stats ----------
            npieces = max(1, n // BN_SUB)
            pw = n // npieces
            stats = [work.tile([P, npieces, 6], F32, tag=f"bnst{j}",
                               name=f"bnst{j}") for j in range(CK)]
            for piece in range(npieces):
                for j in range(CK):
                    sl = slice(piece * pw, (piece + 1) * pw)
                    nc.sync.dma_start(
                        out=x_sb[j][:, sl],
                        in_=x_d[j * P:(j + 1) * P, sl],
                    )
                    nc.vector.bn_stats(out=stats[j][:, piece, :],
                                       in_=x_sb[j][:, sl])
                    nc.vector.tensor_copy(xf8[:, j, sl], x_sb[j][:, sl])
            wq_raw = []
            for i in range(4):  # q,k weight rows 0..512
                t = work.tile([P, C], F32, tag="wraw", name=f"wqraw{i}")
                nc.scalar.dma_start(out=t, in_=wqkv_d[i * P:(i + 1) * P, :])
                wq_raw.append(t)
            wv_raw = []
            for i in range(CK):  # v weight rows 512..768
                t = work.tile([P, C], F32, tag="wvraw", name=f"wvraw{i}")
                nc.scalar.dma_start(out=t, in_=wqkv_d[2 * C + i * P:2 * C + (i + 1) * P, :])
                wv_raw.append(t)
            wo_raw = []
            for i in range(CK):
                t = work.tile([P, C], F32, tag="woraw", name=f"woraw{i}")
                nc.scalar.dma_start(out=t, in_=wout_d[i * P:(i + 1) * P, :])
                wo_raw.append(t)
            for j in range(CK):
                sl = slice(j * P, (j + 1) * P)
                nc.sync.dma_start(out=scale_sb[j], in_=gns_d[sl].rearrange("(a u) -> a u", u=1))
                nc.sync.dma_start(out=bias_sb[j], in_=gnb_d[sl].rearrange("(a u) -> a u", u=1))
                nc.sync.dma_start(out=bq_sb[j], in_=bqkv_d[sl].rearrange("(a u) -> a u", u=1))
                nc.sync.dma_start(
                    out=bk_sb[j],
                    in_=bqkv_d[C + j * P:C + (j + 1) * P].rearrange("(a u) -> a u", u=1))
                nc.sync.dma_start(
                    out=bv_sb[j],
                    in_=bqkv_d[2 * C + j * P:2 * C + (j + 1) * P].rearrange("(a u) -> a u", u=1))
                nc.sync.dma_start(out=bo_sb[j], in_=bout_d[sl].rearrange("(a u) -> a u", u=1))

            # ---------------- constants ----------------
            make_identity(nc, ident_f)
            make_identity(nc, ident_b)
            nc.vector.memset(eps_sb, EPS)
            nc.vector.memset(eln_sb, ESC_BIAS)
            nc.gpsimd.memset(vw_sb[:, :, C:C + VPAD], 0.0)
            nc.gpsimd.memset(vw_sb[:, :, C:C + 1], 1.0)
            # per-chunk local selectors: sel[c, g] = 1/GS where c//GS == g
            for j in range(CK):
                nc.gpsimd.memset(sel[j], 0.0)
                nc.gpsimd.affine_select(
                    out=sel[j], in_=sel[j], compare_op=mybir.AluOpType.is_gt,
                    fill=1.0 / GS, base=1 - GS, pattern=[[-GS, GL]],
                    channel_multiplier=1,
                )
                nc.gpsimd.affine_select(
                    out=sel[j], in_=sel[j], compare_op=mybir.AluOpType.is_ge,
                    fill=0.0, base=0, pattern=[[-GS, GL]], channel_multiplier=1,
                )
                nc.gpsimd.memset(selT[j], 0.0)
                nc.gpsimd.affine_select(
                    out=selT[j], in_=selT[j], compare_op=mybir.AluOpType.is_gt,
                    fill=1.0, base=1 - GS, pattern=[[1, P]], channel_multiplier=-GS,
                )
                nc.gpsimd.affine_select(
                    out=selT[j], in_=selT[j], compare_op=mybir.AluOpType.is_ge,
                    fill=0.0, base=0, pattern=[[1, P]], channel_multiplier=-GS,
                )

            # ------------- weight transposes (PE) + bf16 staging ---------
            for i in range(4):
                for j in range(CK):
                    pt = ps_t.tile([P, P], F32, tag="t", name="wtp")
                    nc.tensor.transpose(pt, wq_raw[i][:, j * P:(j + 1) * P], ident_f)
                    nc.vector.tensor_copy(wqkT_st[j][:, i * P:(i + 1) * P], pt)
            for i in range(CK):
                for j in range(CK):
                    pt = ps_t.tile([P, P], F32, tag="t", name="wtp2")
                    nc.tensor.transpose(pt, wo_raw[i][:, j * P:(j + 1) * P], ident_f)
                    nc.vector.tensor_copy(woT_st[j][:, i * P:(i + 1) * P], pt)
            wv_b = []
            for i in range(CK):
                t = work.tile([P, C], BF16, tag="wvb", name=f"wvb{i}")
                nc.vector.tensor_copy(t, wv_raw[i])
                wv_b.append(t)

            # ------------- Wvo = w_out @ Wv  (bf16, PE) ------------------
            wvo_st = []
            for os_ in range(CK):
                ps = ps_t.tile([P, C], F32, tag="t", name="ps_wvo")
                for mc in range(CK):
                    nc.tensor.matmul(
                        ps, woT_st[mc][:, os_ * P:(os_ + 1) * P], wv_b[mc],
                        start=(mc == 0), stop=(mc == CK - 1),
                    )
                t = work.tile([P, C], BF16, tag="wvo_sb", name=f"wvo{os_}")
                nc.vector.tensor_copy(t, ps)
                wvo_st.append(t)
            for os_ in range(CK):  # WvoT (c-part, o-free) via PE transpose
                for j in range(CK):
                    pt = ps_t.tile([P, P], BF16, tag="t", name="wvtp")
                    nc.tensor.transpose(pt, wvo_st[os_][:, j * P:(j + 1) * P], ident_b)
                    nc.vector.tensor_copy(wvoT_st[j][:, os_ * P:(os_ + 1) * P], pt)

            # ------------- group norm stats -> a_c, beta_c ---------------
            for j in range(CK):
                mv = work.tile([P, 2], F32, tag="mv", name=f"mv{j}")
                nc.vector.bn_aggr(out=mv, in_=stats[j])
                m2 = work.tile([P, 1], F32, tag="m2", name=f"m2{j}")
                nc.vector.tensor_mul(m2, mv[:, 0:1], mv[:, 0:1])
                mv2 = work.tile([P, 2], F32, tag="mv2", name=f"mv2{j}")
                nc.vector.tensor_copy(mv2[:, 0:1], mv[:, 0:1])
                nc.vector.tensor_add(mv2[:, 1:2], mv[:, 1:2], m2)
                ps_g = ps_t.tile([GL, 2], F32, tag="t", name="ps_g")
                nc.tensor.matmul(ps_g, sel[j], mv2, start=True, stop=True)
                gs = work.tile([GL, 2], F32, tag="gs", name=f"gs{j}")
                nc.vector.tensor_copy(gs, ps_g)
                gm2 = work.tile([GL, 1], F32, tag="gm2", name=f"gm2{j}")
                nc.vector.tensor_mul(gm2, gs[:, 0:1], gs[:, 0:1])
                gvar = work.tile([GL, 1], F32, tag="gvar", name=f"gvar{j}")
                nc.vector.tensor_sub(gvar, gs[:, 1:2], gm2)
                gsd = work.tile([GL, 1], F32, tag="gsd", name=f"gsd{j}")
                nc.scalar.activation(out=gsd, in_=gvar,
                                     func=mybir.ActivationFunctionType.Sqrt,
                                     bias=eps_sb, scale=1.0)
                grstd = work.tile([GL, 1], F32, tag="grstd", name=f"grstd{j}")
                nc.vector.reciprocal(grstd, gsd)
                gstat2 = work.tile([GL, 2], F32, tag="gstat2", name=f"gstat2{j}")
                nc.vector.tensor_copy(gstat2[:, 0:1], gs[:, 0:1])
                nc.vector.tensor_copy(gstat2[:, 1:2], grstd)
                ps_bc = ps_t.tile([P, 2], F32, tag="t", name="ps_bc")
                nc.tensor.matmul(ps_bc, selT[j], gstat2, start=True, stop=True)
                nc.vector.tensor_mul(a_c[j], ps_bc[:, 1:2], scale_sb[j])
                t_c = work.tile([P, 1], F32, tag="t_c", name=f"t_c{j}")
                nc.vector.tensor_mul(t_c, ps_bc[:, 0:1], a_c[j])
                nc.vector.tensor_sub(beta_c[j], bias_sb[j], t_c)
                nc.vector.tensor_copy(beta_b16[j], beta_c[j])
                nc.vector.tensor_copy(bv_b16[j], bv_sb[j])

            # preload the exp table set so the first real exp doesn't stall
            dummy_exp = work.tile([1, 1], F32, tag="dummy", name="dummy_exp")
            nc.scalar.activation(out=dummy_exp, in_=eps_sb[0:1, :], func=EXPF)

            # ------------- bias folds (tiny PE matmuls) ------------------
            # bq' = bq + Wq@beta ; bk' = bk + Wk@beta
            for idx, (dst, bsrc) in enumerate([(bqs_sb, bq_sb), (bks_sb, bk_sb)]):
                for oc in range(CK):
                    ps = ps_t.tile([P, 1], F32, tag="t", name="ps_bias")
                    for kc in range(CK):
                        off = idx * C + oc * P
                        nc.tensor.matmul(
                            ps, wqkT_st[kc][:, off:off + P], beta_b16[kc],
                            start=(kc == 0), stop=(kc == CK - 1),
                        )
                    nc.vector.tensor_add(dst[oc], ps, bsrc[oc])
            # symmetric scaling: q and k each carry sqrt(SCALE)=0.25 so both
            # live at sigma~0.25 in fp8 (asymmetric 1/16 on q drowns it in
            # denormals).
            for oc in range(CK):
                nc.vector.tensor_scalar_mul(out=bqs_sb[oc], in0=bqs_sb[oc],
                                            scalar1=SCALE ** 0.5)
                nc.vector.tensor_scalar_mul(out=bks_sb[oc], in0=bks_sb[oc],
                                            scalar1=SCALE ** 0.5)
            # totbias = b_out + w_out@bv + Wvo@beta
            for oc in range(CK):
                ps1 = ps_t.tile([P, 1], F32, tag="t", name="ps_tb1")
                for kc in range(CK):
                    nc.tensor.matmul(
                        ps1, wvoT_st[kc][:, oc * P:(oc + 1) * P], beta_b16[kc],
                        start=(kc == 0), stop=(kc == CK - 1),
                    )
                ps2 = ps_t.tile([P, 1], F32, tag="t", name="ps_tb2")
                for mc in range(CK):
                    nc.tensor.matmul(
                        ps2, woT_st[mc][:, oc * P:(oc + 1) * P], bv_b16[mc],
                        start=(mc == 0), stop=(mc == CK - 1),
                    )
                t1 = work.tile([P, 1], F32, tag="tb1", name="tb1")
                nc.vector.tensor_copy(t1, ps1)
                nc.vector.tensor_add(t1, t1, ps2)
                nc.vector.tensor_add(tb_sb[oc], t1, bo_sb[oc])

            # ------------- GN-folded fp8 weights -------------------------
            # fp8 weights scaled x16: raw w ~ N(0, 1/16) sits in e4m3
            # denormal territory; x16 restores relative precision.  The
            # eviction scalars divide it back out.
            for kc in range(CK):
                nc.vector.tensor_scalar(
                    out=wqkT3[:, kc, :], in0=wqkT_st[kc],
                    scalar1=a_c[kc], scalar2=16.0,
                    op0=mybir.AluOpType.mult, op1=mybir.AluOpType.mult,
                )
                nc.vector.tensor_scalar(
                    out=wvoT3[:, kc, :], in0=wvoT_st[kc],
                    scalar1=a_c[kc], scalar2=16.0,
                    op0=mybir.AluOpType.mult, op1=mybir.AluOpType.mult,
                )

            # ------------- xb = x + totbias (residual + folded bias) -----
            for j in range(CK):
                nc.vector.tensor_scalar_add(out=xb_sb[j], in0=x_sb[j],
                                            scalar1=tb_sb[j])

            # ------------- q, k projections (fp8 DR, FD=512) -------------
            qn = min(n, NB)
            SQ = SCALE ** 0.5 / 16.0   # sqrt(attn scale) / weight-x16
            IDF = mybir.ActivationFunctionType.Identity
            i_qk = 0
            for idx, (dst, sc, bias_ap) in enumerate(
                [(k3, SQ, bks_sb), (q3, SQ, bqs_sb)]
            ):
                for nb in range(n // qn):
                    nsl = slice(nb * qn, (nb + 1) * qn)
                    for oc in range(CK):
                        off = (1 - idx) * C + oc * P  # k first, then q
                        # 4 psums in flight (both rings) + evictions split
                        # DVE/ACT: the 2-slot ping-pong serialized this phase
                        # at ~700ns/matmul before.
                        if i_qk % 2 == 0:
                            ps = ps_s.tile([P, qn], F32, tag="s", name="ps_qk")
                        else:
                            ps = ps_ao.tile([P, qn], F32, tag="ao", name="ps_qk")
                        nc.tensor.matmul(
                            ps, wqkT3[:, :, off:off + P], xf8[:, :, nsl],
                            perf_mode=DR, start=True, stop=True,
                        )
                        if i_qk % 2 == 0:
                            nc.vector.tensor_scalar(
                                out=dst[:, oc, nsl], in0=ps,
                                scalar1=sc, scalar2=bias_ap[oc],
                                op0=mybir.AluOpType.mult, op1=mybir.AluOpType.add,
                            )
                        else:
                            nc.scalar.activation(
                                out=dst[:, oc, nsl], in_=ps, func=IDF,
                                bias=bias_ap[oc], scale=sc,
                            )
                        i_qk += 1

            # ---------------- attention ----------------
            packs = [nsup] * (mtn // nsup)
            if mtn % nsup:
                packs.append(mtn % nsup)
            pack_base = [sum(packs[:i]) for i in range(len(packs))]

            def emit_super(b, st):
                par = b % 2
                sz, base = packs[st], pack_base[st]
                ps = ps_s.tile([P, sz, NB], F32, tag="s", name="ps_s")
                for sub in range(sz):
                    t = base + sub
                    nc.tensor.matmul(
                        ps[:, sub, :],
                        k3[:, :, t * P:(t + 1) * P],
                        q3[:, :, b * NB:(b + 1) * NB],
                        perf_mode=DR, start=True, stop=True,
                    )
                nc.scalar.activation(
                    out=expT[par][:, base:base + sz, :], in_=ps,
                    func=EXPF, bias=eln_sb,
                )

            def emit_vw(mt):
                ps = ps_t.tile([P, C], F32, tag="t", name="ps_vw")
                nc.tensor.matmul(
                    ps, xf8[:, :, mt * P:(mt + 1) * P], wvoT3[:, :, :],
                    perf_mode=DR, start=True, stop=True,
                )
                nc.vector.tensor_scalar_mul(out=vw_sb[:, mt, 0:C], in0=ps,
                                            scalar1=1.0 / 16.0)

            def emit_chain(b, c):
                par = b % 2
                ao = ps_ao.tile([P, NB], F32, tag="ao", name="ps_ao")
                npairs = mtn // 2
                for j in range(npairs):
                    nc.tensor.matmul(
                        ao[:, 0:C + VPAD],
                        expT[par][:, 2 * j:2 * j + 2, c * P:(c + 1) * P],
                        vw_sb[:, 2 * j:2 * j + 2, :],
                        perf_mode=DR, start=(j == 0), stop=(j == npairs - 1),
                    )
                return ao

            def emit_tail(b, c, ao, o_tiles):
                recip = work2.tile([P, 1], F32, tag="recip", name="recip")
                nc.vector.reciprocal(recip, ao[:, C:C + 1])
                aot = work2.tile([P, C], BF16, tag="aot", bufs=4, name="aot")
                nc.vector.tensor_scalar_mul(out=aot, in0=ao[:, 0:C], scalar1=recip)
                for oc in range(CK):
                    pt = ps_t.tile([P, P], BF16, tag="t", name="ao_tp")
                    nc.tensor.transpose(pt, aot[:, oc * P:(oc + 1) * P], ident_b)
                    nc.vector.tensor_add(
                        o_tiles[oc][:, c * P:(c + 1) * P], pt,
                        xb_sb[oc][:, b * NB + c * P:b * NB + (c + 1) * P],
                    )

            def emit_out_dma(b, o_tiles):
                nsl = slice(b * NB, (b + 1) * NB)
                for oc in range(CK):
                    nc.sync.dma_start(out=out_d[oc * P:(oc + 1) * P, nsl],
                                      in_=o_tiles[oc])

            nsubs = NB // P  # 4 query chunks per block
            prev = None
            vw_left = list(range(mtn))  # vw tiles to emit inside block 0
            for b in range(nblk):
                chains = {}
                o_tiles = None
                if prev is not None:
                    o_tiles = [evac.tile([P, NB], F32, tag=f"o{oc}",
                                         name=f"o_sb{oc}") for oc in range(CK)]
                for st in range(len(packs)):
                    emit_super(b, st)
                    if prev is None:
                        # fill the ACT-paced warmup block with the vw projection
                        for _ in range(min(3, len(vw_left))):
                            emit_vw(vw_left.pop(0))
                    else:
                        # interleave: chains early, tails trailing (11 slots)
                        sched = {1: ('c', 0), 3: ('c', 1), 4: ('t', 0),
                                 6: ('c', 2), 7: ('t', 1), 9: ('c', 3),
                                 10: ('t', 2)}
                        if st in sched:
                            kind, c = sched[st]
                            if kind == 'c':
                                chains[c] = emit_chain(prev, c)
                            else:
                                emit_tail(prev, c, chains[c], o_tiles)
                if prev is not None:
                    emit_tail(prev, nsubs - 1, chains[nsubs - 1], o_tiles)
                    emit_out_dma(prev, o_tiles)
                prev, prev_chains = b, chains
            # drain: chains + tails for the final block
            o_tiles = [evac.tile([P, NB], F32, tag=f"o{oc}", name=f"o_sb{oc}")
                       for oc in range(CK)]
            chains = {}
            for c in range(nsubs):
                chains[c] = emit_chain(prev, c)
                if c >= 1:
                    emit_tail(prev, c - 1, chains[c - 1], o_tiles)
            emit_tail(prev, nsubs - 1, chains[nsubs - 1], o_tiles)
            emit_out_dma(prev, o_tiles)

    return nc


_CACHED_NC = {}


def build_nc(n=N):
    if n not in _CACHED_NC:
        nc = bacc.Bacc("TRN2", target_bir_lowering=False, debug=False,
                       num_devices=B)
        build_attention_program(nc, n=n)
        nc.compile()
        _CACHED_NC[n] = nc
    return _CACHED_NC[n]


def make_in_maps(x, gn_scale, gn_bias, w_qkv, b_qkv, w_out, b_out):
    f = np.ascontiguousarray
    return [
        {
            "x": f(x[b].reshape(C, N), dtype=np.float32),
            "gn_scale": f(gn_scale, dtype=np.float32),
            "gn_bias": f(gn_bias, dtype=np.float32),
            "w_qkv": f(w_qkv, dtype=np.float32),
            "b_qkv": f(b_qkv, dtype=np.float32),
            "w_out": f(w_out, dtype=np.float32),
            "b_out": f(b_out, dtype=np.float32),
        }
        for b in range(B)
    ]


def kernel(x, gn_scale, gn_bias, w_qkv, b_qkv, w_out, b_out, _trace=False,
           _tmpdir=None):
    x = np.asarray(x)
    nc = build_nc()
    in_maps = make_in_maps(x, gn_scale, gn_bias, w_qkv, b_qkv, w_out, b_out)
    res = run_bass_kernel_spmd(nc, in_maps, list(range(B)), trace=_trace,
                               tmpdir=_tmpdir)
    out = np.stack([res.results[b]["out"] for b in range(B)])
    out = out.reshape(B, C, H, W).astype(np.float32)
    if _trace:
        kernel.last_exec_time_ns = res.exec_time_ns
        kernel.last_results = res
    return out
